# revision 24
# baseline (speedup 1.0000x reference)
"""AR(1) model kernel for Trainium2, 8-core data parallel.

Computes out[b,t,n,0] = x[b,t-1,n,0]*w + bias for t>=1, out[b,0,n,0] = 0,
for x of shape (64, 288, 2000, 1), w = weights[0,0,0], bias scalar.

Sharding: pure data parallel on batch — 8 batches per core; the scalar
weight/bias are replicated into the program as fp32 immediates (the Bass
program is compiled per (w, bias) value, cached; bit-identical to reading
them from memory).

Per core the work is a shifted scaled copy: for each local batch b, the
574,000-float block x[b, 0:287, :] maps contiguously to out[b, 1:288, :].
574000 = 112*5125, so [112, cols] chunks tile a batch exactly. Loads issue
on the SP (sync) HWDGE ring and stores on the ACT (scalar) ring so a store
waiting on compute never head-of-line blocks the next load.
"""

import numpy as np

import concourse.bacc as bacc
import concourse.mybir as mybir
import concourse.tile as tile
from concourse import bass_utils

B, T, N = 64, 288, 2000
NCORES = 8
BL = B // NCORES          # 8 local batches per core
TN = T * N                # 576000 floats per batch
BODY = (T - 1) * N        # 574000 floats shifted per batch
TOT = BL * TN             # 4608000 floats per core

PART = 112                # 574000 = 112 * 5125
FREE = BODY // PART       # 5125

_nc_cache = {}


def _build_nc(w, bias):
    nc = bacc.Bacc(
        "TRN2", target_bir_lowering=False, debug=False, num_devices=NCORES
    )
    f32 = mybir.dt.float32
    x = nc.dram_tensor("x", [TOT], f32, kind="ExternalInput").ap()
    out = nc.dram_tensor("out", [TOT], f32, kind="ExternalOutput").ap()

    with tile.TileContext(nc) as tc:
        with (
            tc.tile_pool(name="consts", bufs=1) as consts,
            tc.tile_pool(name="data", bufs=18) as data,
        ):
            zt = consts.tile([BL, N], f32)
            nc.vector.memset(zt[:], 0.0)

            def piece(xo, oo, part, free, col0, cols, store_eng=None):
                """load/scale/store one [part, cols] slice of a batch."""
                t0 = data.tile([part, cols], f32, tag="t0")
                src = x[xo : xo + part * free].rearrange("(p f) -> p f", p=part)
                dst = out[oo : oo + part * free].rearrange("(p f) -> p f", p=part)
                nc.sync.dma_start(t0[:], src[:, col0 : col0 + cols])
                nc.vector.tensor_scalar(
                    t0[:],
                    t0[:],
                    w,
                    bias,
                    mybir.AluOpType.mult,
                    mybir.AluOpType.add,
                )
                (store_eng or nc.scalar).dma_start(
                    dst[:, col0 : col0 + cols], t0[:]
                )

            # ~1.15MB half-batch chunks in the steady state. The opening
            # chunk is small so the first store engages the write channel
            # early; the final batch is split with its last store issued on
            # the (by then idle) sync ring so the two final stores drain in
            # parallel.
            for b in range(BL):
                xo, oo = b * TN, b * TN + N
                if b == 0:
                    piece(xo, oo, PART, FREE, 0, 640)
                    piece(xo, oo, PART, FREE, 640, 1925)
                    piece(xo, oo, PART, FREE, 2565, 2560)
                elif b == BL - 1:
                    piece(xo, oo, PART, FREE, 0, 2565)
                    piece(xo, oo, PART, FREE, 2565, 1280)
                    piece(xo, oo, PART, FREE, 3845, 1280, store_eng=nc.sync)
                else:
                    piece(xo, oo, PART, FREE, 0, 2565)
                    piece(xo, oo, PART, FREE, 2565, 2560)

            # Zero rows t=0 of every local batch: one strided [BL, N]
            # store, issued at the tail of the load (sync) ring where it
            # overlaps the remaining big stores instead of the fill phase.
            out2d = out.rearrange("(b q) -> b q", b=BL)
            nc.sync.dma_start(out2d[:, 0:N], zt[:])

    nc.compile()
    return nc


def _get_nc(w, bias):
    key = (float(w), float(bias))
    if key not in _nc_cache:
        _nc_cache[key] = _build_nc(*key)
    return _nc_cache[key]


def kernel(x, weights, bias, _trace=False):
    x = np.ascontiguousarray(np.asarray(x, dtype=np.float32)).reshape(B, TN)
    w_val = np.float32(np.asarray(weights).reshape(-1)[0])
    b_val = np.float32(np.asarray(bias).reshape(-1)[0])
    in_maps = [{"x": x[c * BL : (c + 1) * BL].reshape(-1)} for c in range(NCORES)]
    nc = _get_nc(w_val, b_val)
    try:
        res = bass_utils.run_bass_kernel_spmd(
            nc, in_maps, core_ids=list(range(NCORES)), trace=_trace
        )
    except Exception:
        # The axon worker occasionally reports a transient device error;
        # one retry after a pause recovers when the fault is per-execution.
        import time

        time.sleep(5)
        res = bass_utils.run_bass_kernel_spmd(
            nc, in_maps, core_ids=list(range(NCORES)), trace=_trace
        )
    out = np.concatenate(
        [res.results[c]["out"].reshape(BL, T, N, 1) for c in range(NCORES)], axis=0
    )
    if _trace:
        return out, res
    return out


# revision 25
# speedup vs baseline: 1.0538x; 1.0538x over previous
"""AR(1) model kernel for Trainium2, 8-core data parallel.

Computes out[b,t,n,0] = x[b,t-1,n,0]*w + bias for t>=1, out[b,0,n,0] = 0,
for x of shape (64, 288, 2000, 1), w = weights[0,0,0], bias scalar.

Sharding: pure data parallel on batch — 8 batches per core; the scalar
weight/bias are replicated into the program as fp32 immediates (the Bass
program is compiled per (w, bias) value, cached; bit-identical to reading
them from memory).

Per core the work is a shifted scaled copy: for each local batch b, the
574,000-float block x[b, 0:287, :] maps contiguously to out[b, 1:288, :].
574000 = 112*5125, so [112, cols] chunks tile a batch exactly. Loads issue
on the SP (sync) HWDGE ring and stores on the ACT (scalar) ring so a store
waiting on compute never head-of-line blocks the next load.
"""

import numpy as np

import concourse.bacc as bacc
import concourse.mybir as mybir
import concourse.tile as tile
from concourse import bass_utils

B, T, N = 64, 288, 2000
NCORES = 8
BL = B // NCORES          # 8 local batches per core
TN = T * N                # 576000 floats per batch
BODY = (T - 1) * N        # 574000 floats shifted per batch
TOT = BL * TN             # 4608000 floats per core

PART = 112                # 574000 = 112 * 5125
FREE = BODY // PART       # 5125

_nc_cache = {}


def _build_nc(w, bias):
    nc = bacc.Bacc(
        "TRN2", target_bir_lowering=False, debug=False, num_devices=NCORES
    )
    f32 = mybir.dt.float32
    x = nc.dram_tensor("x", [TOT], f32, kind="ExternalInput").ap()
    out = nc.dram_tensor("out", [TOT], f32, kind="ExternalOutput").ap()

    with tile.TileContext(nc) as tc:
        with (
            tc.tile_pool(name="consts", bufs=1) as consts,
            tc.tile_pool(name="data", bufs=18) as data,
        ):
            # Zero rows t=0 of every local batch: one strided [BL, N] store,
            # issued on the store ring before the big stores queue up.
            zt = consts.tile([BL, N], f32)
            nc.vector.memset(zt[:], 0.0)
            out2d = out.rearrange("(b q) -> b q", b=BL)
            nc.scalar.dma_start(out2d[:, 0:N], zt[:])

            def piece(xo, oo, part, free, col0, cols):
                """load/scale/store one [part, cols] slice of a batch."""
                t0 = data.tile([part, cols], f32, tag="t0")
                src = x[xo : xo + part * free].rearrange("(p f) -> p f", p=part)
                dst = out[oo : oo + part * free].rearrange("(p f) -> p f", p=part)
                nc.sync.dma_start(t0[:], src[:, col0 : col0 + cols])
                nc.vector.tensor_scalar(
                    t0[:],
                    t0[:],
                    w,
                    bias,
                    mybir.AluOpType.mult,
                    mybir.AluOpType.add,
                )
                nc.scalar.dma_start(dst[:, col0 : col0 + cols], t0[:])

            # ~1.15MB half-batch chunks: small enough to fill/drain the
            # load-store pipeline quickly, large enough for good DMA
            # efficiency; the opening chunks are tapered smaller so the
            # first store engages the write channel early.
            for b in range(BL):
                xo, oo = b * TN, b * TN + N
                if b == 0:
                    piece(xo, oo, PART, FREE, 0, 1285)
                    piece(xo, oo, PART, FREE, 1285, 1280)
                    piece(xo, oo, PART, FREE, 2565, 2560)
                else:
                    piece(xo, oo, PART, FREE, 0, 2565)
                    piece(xo, oo, PART, FREE, 2565, 2560)

    nc.compile()
    return nc


def _get_nc(w, bias):
    key = (float(w), float(bias))
    if key not in _nc_cache:
        _nc_cache[key] = _build_nc(*key)
    return _nc_cache[key]


def kernel(x, weights, bias, _trace=False):
    x = np.ascontiguousarray(np.asarray(x, dtype=np.float32)).reshape(B, TN)
    w_val = np.float32(np.asarray(weights).reshape(-1)[0])
    b_val = np.float32(np.asarray(bias).reshape(-1)[0])
    in_maps = [{"x": x[c * BL : (c + 1) * BL].reshape(-1)} for c in range(NCORES)]
    nc = _get_nc(w_val, b_val)
    try:
        res = bass_utils.run_bass_kernel_spmd(
            nc, in_maps, core_ids=list(range(NCORES)), trace=_trace
        )
    except Exception:
        # The axon worker occasionally reports a transient device error;
        # one retry after a pause recovers when the fault is per-execution.
        import time

        time.sleep(5)
        res = bass_utils.run_bass_kernel_spmd(
            nc, in_maps, core_ids=list(range(NCORES)), trace=_trace
        )
    out = np.concatenate(
        [res.results[c]["out"].reshape(BL, T, N, 1) for c in range(NCORES)], axis=0
    )
    if _trace:
        return out, res
    return out
